# revision 30
# baseline (speedup 1.0000x reference)
"""Contrastive-loss kernel for Trainium2 (Bass/Tile), 8-core SPMD.

Reference semantics (B=4, N=4096, D=128, T=0.1):
    emb_n = emb / max(||emb||, 1e-12)
    pos_sim[b,n] = dot(emb_n[b,n], emb_n[b, pos_idx[b,n]]) / T
    loss = mean(softplus(-pos_sim)) + mean(softplus(neg_sim))

Each of the 8 cores handles half the rows of one batch element.  It loads a
single merged [128, 256] i16 index tile and its 2048 own rows (bf16), then
issues 4 dma_gathers (1024 rows each) in interleaved direction order
(pos0, neg0, pos1, neg1) so each slice's softplus accumulation can retire as
soon as its pair of gathers lands.  Per unit:

    z    = -+ dot(own, g) * exp(-0.5*(ln ssq_own + ln ssq_g)) / T
    part = softplus(z) = ln(exp(z) + 1)

Squares are split between ACT and DVE, group-of-128 row sums are 3-level
pairwise-add trees + short reduce on DVE with selected trees on the
Pool/GPSIMD engine, pos dots negated in their tree so one Exp serves both
softplus directions, and the +1 folds into the final Ln's bias, which
accumulates into one output column per slice.  tile_wait_until staging keys
the Tile scheduler to each gather's arrival so own-row work fills the early
window.  Output is [128, 2] f32 per core; the host sums / (B*N).
"""

import numpy as np

B, N, D = 4, 4096, 128
NCORES = 8
HALF = N // 2        # rows per core
CHUNK = HALF // 128  # 16 column-groups of 128
TEMP = 0.1
NSPLIT = 2           # slices per direction
SC = CHUNK // NSPLIT # column-groups per slice (8)
SW = SC * D          # free-dim elements per slice (1024)
NIDX = HALF // NSPLIT

# engine assignment knobs (tuned against TimelineSim)
SQ_ENG = {"o0": "dve", "o1": "act", "p0": "act", "p1": "act",
          "n0": "act", "n1": "act"}
TREE_ENG = {"o0": "dve", "o1": "dve", "p0": "dve", "p1": "pool",
            "n0": "pool", "n1": "dve",
            "dp0": "pool", "dp1": "dve", "dn0": "pool", "dn1": "dve"}
COMB_TT_ENG = "pool"   # engine for the small sprod/cosz tensor_tensors
# gather + compute order and scheduler staging (ms ranks)
UNIT_ORDER = [("p", 0), ("n", 0), ("p", 1), ("n", 1)]
UNIT_TS = {"p0": 0.0074, "n0": 0.0089, "p1": 0.0103, "n1": 0.0118}
UNIT_NSUB = {}          # split late units into finer column-chunks

_PROG = None


def _pin_act_table(table_name="natural_log_exp_and_others"):
    """Make Square/Ln/Exp resolve only to `table_name` so the act-table-load
    pass emits a single table load instead of ping-ponging between tables."""
    import functools
    import concourse.hw_specs as hw_specs
    import concourse.bacc as bacc
    import concourse.mybir as mybir

    if getattr(_pin_act_table, "_done", False):
        return
    orig = hw_specs.get_activation_tables
    AF = mybir.ActivationFunctionType
    pinned = {AF.Square, AF.Ln, AF.Exp}

    @functools.cache
    def patched(arch):
        return {k: (v if k == table_name else v - pinned)
                for k, v in orig(arch).items()}

    hw_specs.get_activation_tables = patched
    bacc.get_activation_tables = patched
    _pin_act_table._done = True


def _build_program():
    import concourse.bacc as bacc
    import concourse.tile as tile
    import concourse.mybir as mybir

    _pin_act_table()

    f32 = mybir.dt.float32
    bf16 = mybir.dt.bfloat16
    i16 = mybir.dt.int16
    mult = mybir.AluOpType.mult
    add = mybir.AluOpType.add
    X = mybir.AxisListType.X
    AF = mybir.ActivationFunctionType

    nc = bacc.Bacc("TRN2", target_bir_lowering=False)

    table = nc.dram_tensor("table", [N, D], bf16, kind="ExternalInput")
    own = nc.dram_tensor("own", [128, HALF], bf16, kind="ExternalInput")
    idx = nc.dram_tensor("idx", [128, 2 * 128], i16, kind="ExternalInput")
    out = nc.dram_tensor("partial", [128, NSPLIT], f32, kind="ExternalOutput")

    with tile.TileContext(nc) as tc:
        with tc.tile_pool(name="p", bufs=1) as pool:
            idx_t = pool.tile([128, 2 * 128], i16)
            nc.sync.dma_start(out=idx_t[:], in_=idx[:])
            own_t = pool.tile([128, HALF], bf16)
            nc.sync.dma_start(out=own_t[:], in_=own[:])

            # gathers in UNIT_ORDER; idx cols: pos slice s -> s*64, neg -> 128+s*64
            gath = {}
            for d, s in UNIT_ORDER:
                u = f"{d}{s}"
                icol = (0 if d == "p" else 128) + s * 64
                g = pool.tile([128, SW], bf16, tag=f"g{u}", name=f"g{u}")
                nc.gpsimd.dma_gather(
                    out_ap=g[:].rearrange("p (c d) -> p c d", d=D),
                    in_ap=table[:],
                    idxs_ap=idx_t[:, icol:icol + 64],
                    num_idxs=NIDX,
                    num_idxs_reg=NIDX,
                    elem_size=D,
                    single_packet=False,
                )
                gath[u] = g

            def eng(which):
                return {"dve": nc.vector, "pool": nc.gpsimd}[which]

            def base_site(site):
                return site.split("_")[0]

            def square(site, src_ap):
                """x^2 -> bf16 tile, on ACT (activation) or DVE (x*x)."""
                sq = pool.tile(list(src_ap.shape), bf16, tag=f"sq{site}",
                               name=f"sq{site}")
                if SQ_ENG[base_site(site)] == "act":
                    nc.scalar.square(sq[:], src_ap)
                else:
                    eng(SQ_ENG[base_site(site)]).tensor_tensor(
                        out=sq[:], in0=src_ap, in1=src_ap, op=mult)
                return sq

            deferred = []   # (site, closure) for pool-tree final reduces

            def group_sum(site, src_ap, out_ap, negate=False, width=SC):
                """[128, width*D] bf16 -> [128, width] f32 row-group sums.
                3 pairwise-add tree levels on TREE_ENG[site], short reduce on
                DVE.  Pool-tree reduces are deferred so they don't head-block
                the in-order DVE queue while the (slower) Pool tree runs."""
                on_pool = TREE_ENG[base_site(site)] == "pool"
                e = eng("pool" if on_pool else "dve")
                w = D
                cur = src_ap
                for lvl in range(3):
                    t = pool.tile([128, width * (w // 2)], bf16,
                                  tag=f"t{site}l{lvl}", name=f"t{site}l{lvl}")
                    v = cur.rearrange("p (c d) -> p c d", d=w)
                    e.tensor_tensor(
                        out=t[:].rearrange("p (c d) -> p c d", d=w // 2),
                        in0=v[:, :, 0:w // 2], in1=v[:, :, w // 2:w], op=add)
                    cur = t[:]
                    w //= 2

                def emit_reduce(cur=cur, w=w, out_ap=out_ap, negate=negate):
                    nc.vector.tensor_reduce(
                        out=out_ap, in_=cur.rearrange("p (c d) -> p c d", d=w),
                        axis=X, op=add, negate=negate)

                if on_pool:
                    deferred.append(emit_reduce)
                else:
                    emit_reduce()

            def flush_deferred():
                for f in deferred:
                    f()
                deferred.clear()

            # own ssq, both slices up front (own lands before any gather)
            ssqo = []
            for s in range(NSPLIT):
                o_ap = own_t[:, s * SW:(s + 1) * SW]
                sq = square(f"o{s}", o_ap)
                r = pool.tile([128, SC], f32, tag=f"ssqo{s}", name=f"ssqo{s}")
                group_sum(f"o{s}", sq[:], r[:])
                ssqo.append(r[:])

            out_t = pool.tile([128, NSPLIT], f32)

            # per-slice combined [pos|neg] tiles; filled as units complete
            ssq_b = [pool.tile([128, 2 * SC], f32, tag=f"ssqb{s}",
                               name=f"ssqb{s}") for s in range(NSPLIT)]
            dot_b = [pool.tile([128, 2 * SC], f32, tag=f"dotb{s}",
                               name=f"dotb{s}") for s in range(NSPLIT)]

            def unit(d, s, nsub=1):
                """ssq + dot for gathered unit (direction d, slice s), in
                nsub column-chunks so late units pipeline finer."""
                u = f"{d}{s}"
                i = 0 if d == "p" else 1
                g = gath[u]
                w = SW // nsub          # free elems per sub-chunk
                c = SC // nsub          # column-groups per sub-chunk
                for k in range(nsub):
                    uk = u if nsub == 1 else f"{u}_{k}"
                    o_ap = own_t[:, s * SW + k * w:s * SW + (k + 1) * w]
                    g_ap = g[:, k * w:(k + 1) * w]
                    pr = pool.tile([128, w], bf16, tag=f"pr{uk}",
                                   name=f"pr{uk}")
                    nc.vector.tensor_tensor(out=pr[:], in0=o_ap, in1=g_ap,
                                            op=mult)
                    sq = square(uk, g_ap)
                    group_sum(uk, sq[:],
                              ssq_b[s][:, i * SC + k * c:i * SC + (k + 1) * c],
                              width=c)
                    group_sum(f"d{uk}", pr[:],
                              dot_b[s][:, i * SC + k * c:i * SC + (k + 1) * c],
                              negate=(d == "p"), width=c)

            def combine(s):
                """softplus accumulation for slice s into out_t[:, s]."""
                sprod = pool.tile([128, 2 * SC], f32, tag=f"sprod{s}",
                                  name=f"sprod{s}")
                for i in range(2):
                    eng(COMB_TT_ENG).tensor_tensor(
                        out=sprod[:, i * SC:(i + 1) * SC],
                        in0=ssqo[s], in1=ssq_b[s][:, i * SC:(i + 1) * SC],
                        op=mult)
                lnp = pool.tile([128, 2 * SC], f32, tag=f"lnp{s}",
                                name=f"lnp{s}")
                nc.scalar.activation(lnp[:], sprod[:], AF.Ln)
                rinv = pool.tile([128, 2 * SC], f32, tag=f"rinv{s}",
                                 name=f"rinv{s}")
                nc.scalar.activation(rinv[:], lnp[:], AF.Exp, scale=-0.5)
                cosz = pool.tile([128, 2 * SC], f32, tag=f"cosz{s}",
                                 name=f"cosz{s}")
                eng(COMB_TT_ENG).tensor_tensor(
                    out=cosz[:], in0=dot_b[s][:], in1=rinv[:], op=mult)
                ez = pool.tile([128, 2 * SC], f32, tag=f"ez{s}",
                               name=f"ez{s}")
                nc.scalar.activation(ez[:], cosz[:], AF.Exp, scale=1.0 / TEMP)
                sp = pool.tile([128, 2 * SC], f32, tag=f"sp{s}",
                               name=f"sp{s}")
                nc.scalar.activation(sp[:], ez[:], AF.Ln, bias=1.0,
                                     accum_out=out_t[:, s:s + 1])

            # emit all units' tree work first, then the deferred pool-tree
            # reduces, then the combines (which consume them)
            for d, s in UNIT_ORDER:
                with tc.tile_wait_until(UNIT_TS[f"{d}{s}"]):
                    unit(d, s, nsub=UNIT_NSUB.get(f"{d}{s}", 1))
            with tc.tile_wait_until(UNIT_TS["n1"] + 0.0004):
                flush_deferred()
                for s in range(NSPLIT):
                    combine(s)

            nc.sync.dma_start(out=out[:], in_=out_t[:])

    nc.compile()
    return nc


def _get_program():
    global _PROG
    if _PROG is None:
        _PROG = _build_program()
    return _PROG


def _wrap_idx(rows):
    """Host-side index layout for dma_gather (one direction -> [128, 128]).

    rows[n] is the partner row for local own-row n (n = p*CHUNK + t in the
    on-chip layout).  Each gather slice s covers chunks [s*SC, (s+1)*SC) and
    reads idx tile columns [s*(128/NSPLIT), ...).  Within a slice,
    dma_gather places gathered row i at partition i%128, chunk i//128, and
    the Q7 cores read the slice's index columns wrapped into 16 partitions
    (idxs[pi, col] = unwrapped[col*16 + pi]) replicated across the 8
    16-partition groups."""
    cols = []
    ncol = 128 // NSPLIT
    for s in range(NSPLIT):
        sl = rows.reshape(128, CHUNK)[:, s * SC:(s + 1) * SC]   # [128, SC]
        unwrapped = sl.T.ravel()                                # [SC*128]
        cols.append(unwrapped.reshape(ncol, 16).T)              # [16, ncol]
    wrapped = np.concatenate(cols, axis=1).astype(np.int16)     # [16, 128]
    return np.tile(wrapped, (8, 1))                             # [128, 128]


def _shard_inputs(embeddings, positive_pairs, negative_pairs):
    import ml_dtypes

    emb = np.asarray(embeddings, dtype=np.float32)
    emb_bf = emb.astype(ml_dtypes.bfloat16)
    pos = np.asarray(positive_pairs).reshape(B, N)
    neg = np.asarray(negative_pairs).reshape(B, N)

    in_maps = []
    for c in range(NCORES):
        b, h = divmod(c, 2)
        own_rows = emb_bf[b, h * HALF:(h + 1) * HALF]       # [HALF, D]
        idx = np.concatenate(
            [_wrap_idx(pos[b, h * HALF:(h + 1) * HALF]),
             _wrap_idx(neg[b, h * HALF:(h + 1) * HALF])], axis=1)
        in_maps.append({
            "table": np.ascontiguousarray(emb_bf[b]),
            "own": np.ascontiguousarray(own_rows.reshape(128, CHUNK * D)),
            "idx": np.ascontiguousarray(idx),
        })
    return in_maps


def kernel(embeddings, positive_pairs, negative_pairs):
    from concourse.bass_utils import run_bass_kernel_spmd

    nc = _get_program()
    in_maps = _shard_inputs(embeddings, positive_pairs, negative_pairs)
    res = run_bass_kernel_spmd(nc, in_maps, core_ids=list(range(NCORES)))
    total = sum(r["partial"].astype(np.float64).sum() for r in res.results)
    return np.float32(total / (B * N))


# revision 31
# speedup vs baseline: 1.1138x; 1.1138x over previous
"""Contrastive-loss kernel for Trainium2 (Bass/Tile), 8-core SPMD.

Reference semantics (B=4, N=4096, D=128, T=0.1):
    emb_n = emb / max(||emb||, 1e-12)
    pos_sim[b,n] = dot(emb_n[b,n], emb_n[b, pos_idx[b,n]]) / T
    loss = mean(softplus(-pos_sim)) + mean(softplus(neg_sim))

Each of the 8 cores handles half the rows of one batch element.  It loads a
single merged [128, 256] i16 index tile and its 2048 own rows (bf16), then
issues 4 dma_gathers (1024 rows each) in interleaved direction order
(pos0, neg0, pos1, neg1) so each slice's softplus accumulation can retire as
soon as its pair of gathers lands.  Per unit:

    z    = -+ dot(own, g) * exp(-0.5*(ln ssq_own + ln ssq_g)) / T
    part = softplus(z) = ln(exp(z) + 1)

Squares are split between ACT and DVE, group-of-128 row sums are 3-level
pairwise-add trees + short reduce on DVE with selected trees on the
Pool/GPSIMD engine, pos dots negated in their tree so one Exp serves both
softplus directions, and the +1 folds into the final Ln's bias, which
accumulates into one output column per slice.  tile_wait_until staging keys
the Tile scheduler to each gather's arrival so own-row work fills the early
window.  Output is [128, 2] f32 per core; the host sums / (B*N).
"""

import numpy as np

B, N, D = 4, 4096, 128
NCORES = 8
HALF = N // 2        # rows per core
CHUNK = HALF // 128  # 16 column-groups of 128
TEMP = 0.1
NSPLIT = 2           # slices per direction
SC = CHUNK // NSPLIT # column-groups per slice (8)
SW = SC * D          # free-dim elements per slice (1024)
NIDX = HALF // NSPLIT

# engine assignment knobs (tuned against TimelineSim)
SQ_ENG = {"o0": "dve", "o1": "act", "p0": "act", "p1": "act",
          "n0": "act", "n1": "act"}
TREE_ENG = {"o0": "dve", "o1": "dve", "p0": "dve", "p1": "pool",
            "n0": "pool", "n1": "dve",
            "dp0": "pool", "dp1": "dve", "dn0": "dve", "dn1": "dve"}
COMB_TT_ENG = "pool"   # engine for the small sprod/cosz tensor_tensors
# gather + compute order and scheduler staging (ms ranks)
UNIT_ORDER = [("p", 0), ("n", 0), ("p", 1), ("n", 1)]
UNIT_TS = {"p0": 0.0074, "n0": 0.0089, "p1": 0.0103, "n1": 0.0118}
UNIT_NSUB = {}          # split late units into finer column-chunks

_PROG = None


def _pin_act_table(table_name="natural_log_exp_and_others"):
    """Make Square/Ln/Exp resolve only to `table_name` so the act-table-load
    pass emits a single table load instead of ping-ponging between tables."""
    import functools
    import concourse.hw_specs as hw_specs
    import concourse.bacc as bacc
    import concourse.mybir as mybir

    if getattr(_pin_act_table, "_done", False):
        return
    orig = hw_specs.get_activation_tables
    AF = mybir.ActivationFunctionType
    pinned = {AF.Square, AF.Ln, AF.Exp}

    @functools.cache
    def patched(arch):
        return {k: (v if k == table_name else v - pinned)
                for k, v in orig(arch).items()}

    hw_specs.get_activation_tables = patched
    bacc.get_activation_tables = patched
    _pin_act_table._done = True


def _build_program():
    import concourse.bacc as bacc
    import concourse.tile as tile
    import concourse.mybir as mybir

    _pin_act_table()

    f32 = mybir.dt.float32
    bf16 = mybir.dt.bfloat16
    i16 = mybir.dt.int16
    mult = mybir.AluOpType.mult
    add = mybir.AluOpType.add
    X = mybir.AxisListType.X
    AF = mybir.ActivationFunctionType

    nc = bacc.Bacc("TRN2", target_bir_lowering=False)

    table = nc.dram_tensor("table", [N, D], bf16, kind="ExternalInput")
    own = nc.dram_tensor("own", [128, HALF], bf16, kind="ExternalInput")
    idx = nc.dram_tensor("idx", [128, 2 * 128], i16, kind="ExternalInput")
    out = nc.dram_tensor("partial", [128, NSPLIT], f32, kind="ExternalOutput")

    with tile.TileContext(nc) as tc:
        with tc.tile_pool(name="p", bufs=1) as pool:
            idx_t = pool.tile([128, 2 * 128], i16)
            nc.sync.dma_start(out=idx_t[:], in_=idx[:])
            own_t = pool.tile([128, HALF], bf16)
            nc.sync.dma_start(out=own_t[:], in_=own[:])

            # gathers in UNIT_ORDER; idx cols: pos slice s -> s*64, neg -> 128+s*64
            gath = {}
            for d, s in UNIT_ORDER:
                u = f"{d}{s}"
                icol = (0 if d == "p" else 128) + s * 64
                g = pool.tile([128, SW], bf16, tag=f"g{u}", name=f"g{u}")
                nc.gpsimd.dma_gather(
                    out_ap=g[:].rearrange("p (c d) -> p c d", d=D),
                    in_ap=table[:],
                    idxs_ap=idx_t[:, icol:icol + 64],
                    num_idxs=NIDX,
                    num_idxs_reg=NIDX,
                    elem_size=D,
                    single_packet=False,
                )
                gath[u] = g

            def eng(which):
                return {"dve": nc.vector, "pool": nc.gpsimd}[which]

            def base_site(site):
                return site.split("_")[0]

            def square(site, src_ap):
                """x^2 -> bf16 tile, on ACT (activation) or DVE (x*x)."""
                sq = pool.tile(list(src_ap.shape), bf16, tag=f"sq{site}",
                               name=f"sq{site}")
                if SQ_ENG[base_site(site)] == "act":
                    nc.scalar.square(sq[:], src_ap)
                else:
                    eng(SQ_ENG[base_site(site)]).tensor_tensor(
                        out=sq[:], in0=src_ap, in1=src_ap, op=mult)
                return sq

            deferred = []   # (site, closure) for pool-tree final reduces

            def group_sum(site, src_ap, out_ap, negate=False, width=SC):
                """[128, width*D] bf16 -> [128, width] f32 row-group sums.
                3 pairwise-add tree levels on TREE_ENG[site], short reduce on
                DVE.  Pool-tree reduces are deferred so they don't head-block
                the in-order DVE queue while the (slower) Pool tree runs."""
                on_pool = TREE_ENG[base_site(site)] == "pool"
                e = eng("pool" if on_pool else "dve")
                w = D
                cur = src_ap
                for lvl in range(3):
                    t = pool.tile([128, width * (w // 2)], bf16,
                                  tag=f"t{site}l{lvl}", name=f"t{site}l{lvl}")
                    v = cur.rearrange("p (c d) -> p c d", d=w)
                    e.tensor_tensor(
                        out=t[:].rearrange("p (c d) -> p c d", d=w // 2),
                        in0=v[:, :, 0:w // 2], in1=v[:, :, w // 2:w], op=add)
                    cur = t[:]
                    w //= 2

                def emit_reduce(cur=cur, w=w, out_ap=out_ap, negate=negate):
                    nc.vector.tensor_reduce(
                        out=out_ap, in_=cur.rearrange("p (c d) -> p c d", d=w),
                        axis=X, op=add, negate=negate)

                if on_pool:
                    deferred.append(emit_reduce)
                else:
                    emit_reduce()

            def flush_deferred():
                for f in deferred:
                    f()
                deferred.clear()

            # own ssq, both slices up front (own lands before any gather)
            ssqo = []
            for s in range(NSPLIT):
                o_ap = own_t[:, s * SW:(s + 1) * SW]
                sq = square(f"o{s}", o_ap)
                r = pool.tile([128, SC], f32, tag=f"ssqo{s}", name=f"ssqo{s}")
                group_sum(f"o{s}", sq[:], r[:])
                ssqo.append(r[:])

            out_t = pool.tile([128, NSPLIT], f32)

            # per-slice combined [pos|neg] tiles; filled as units complete
            ssq_b = [pool.tile([128, 2 * SC], f32, tag=f"ssqb{s}",
                               name=f"ssqb{s}") for s in range(NSPLIT)]
            dot_b = [pool.tile([128, 2 * SC], f32, tag=f"dotb{s}",
                               name=f"dotb{s}") for s in range(NSPLIT)]

            def unit(d, s, nsub=1):
                """ssq + dot for gathered unit (direction d, slice s), in
                nsub column-chunks so late units pipeline finer."""
                u = f"{d}{s}"
                i = 0 if d == "p" else 1
                g = gath[u]
                w = SW // nsub          # free elems per sub-chunk
                c = SC // nsub          # column-groups per sub-chunk
                for k in range(nsub):
                    uk = u if nsub == 1 else f"{u}_{k}"
                    o_ap = own_t[:, s * SW + k * w:s * SW + (k + 1) * w]
                    g_ap = g[:, k * w:(k + 1) * w]
                    pr = pool.tile([128, w], bf16, tag=f"pr{uk}",
                                   name=f"pr{uk}")
                    nc.vector.tensor_tensor(out=pr[:], in0=o_ap, in1=g_ap,
                                            op=mult)
                    sq = square(uk, g_ap)
                    group_sum(uk, sq[:],
                              ssq_b[s][:, i * SC + k * c:i * SC + (k + 1) * c],
                              width=c)
                    group_sum(f"d{uk}", pr[:],
                              dot_b[s][:, i * SC + k * c:i * SC + (k + 1) * c],
                              negate=(d == "p"), width=c)

            def combine(s):
                """softplus accumulation for slice s into out_t[:, s]."""
                sprod = pool.tile([128, 2 * SC], f32, tag=f"sprod{s}",
                                  name=f"sprod{s}")
                for i in range(2):
                    eng(COMB_TT_ENG).tensor_tensor(
                        out=sprod[:, i * SC:(i + 1) * SC],
                        in0=ssqo[s], in1=ssq_b[s][:, i * SC:(i + 1) * SC],
                        op=mult)
                lnp = pool.tile([128, 2 * SC], f32, tag=f"lnp{s}",
                                name=f"lnp{s}")
                nc.scalar.activation(lnp[:], sprod[:], AF.Ln)
                rinv = pool.tile([128, 2 * SC], f32, tag=f"rinv{s}",
                                 name=f"rinv{s}")
                nc.scalar.activation(rinv[:], lnp[:], AF.Exp, scale=-0.5)
                cosz = pool.tile([128, 2 * SC], f32, tag=f"cosz{s}",
                                 name=f"cosz{s}")
                eng(COMB_TT_ENG).tensor_tensor(
                    out=cosz[:], in0=dot_b[s][:], in1=rinv[:], op=mult)
                ez = pool.tile([128, 2 * SC], f32, tag=f"ez{s}",
                               name=f"ez{s}")
                nc.scalar.activation(ez[:], cosz[:], AF.Exp, scale=1.0 / TEMP)
                sp = pool.tile([128, 2 * SC], f32, tag=f"sp{s}",
                               name=f"sp{s}")
                nc.scalar.activation(sp[:], ez[:], AF.Ln, bias=1.0,
                                     accum_out=out_t[:, s:s + 1])

            # emit all units' tree work first, then the deferred pool-tree
            # reduces, then the combines (which consume them)
            for d, s in UNIT_ORDER:
                with tc.tile_wait_until(UNIT_TS[f"{d}{s}"]):
                    unit(d, s, nsub=UNIT_NSUB.get(f"{d}{s}", 1))
            with tc.tile_wait_until(UNIT_TS["n1"] + 0.0004):
                flush_deferred()
                for s in range(NSPLIT):
                    combine(s)

            nc.sync.dma_start(out=out[:], in_=out_t[:])

    nc.compile()
    return nc


def _get_program():
    global _PROG
    if _PROG is None:
        _PROG = _build_program()
    return _PROG


def _wrap_idx(rows):
    """Host-side index layout for dma_gather (one direction -> [128, 128]).

    rows[n] is the partner row for local own-row n (n = p*CHUNK + t in the
    on-chip layout).  Each gather slice s covers chunks [s*SC, (s+1)*SC) and
    reads idx tile columns [s*(128/NSPLIT), ...).  Within a slice,
    dma_gather places gathered row i at partition i%128, chunk i//128, and
    the Q7 cores read the slice's index columns wrapped into 16 partitions
    (idxs[pi, col] = unwrapped[col*16 + pi]) replicated across the 8
    16-partition groups."""
    cols = []
    ncol = 128 // NSPLIT
    for s in range(NSPLIT):
        sl = rows.reshape(128, CHUNK)[:, s * SC:(s + 1) * SC]   # [128, SC]
        unwrapped = sl.T.ravel()                                # [SC*128]
        cols.append(unwrapped.reshape(ncol, 16).T)              # [16, ncol]
    wrapped = np.concatenate(cols, axis=1).astype(np.int16)     # [16, 128]
    return np.tile(wrapped, (8, 1))                             # [128, 128]


def _shard_inputs(embeddings, positive_pairs, negative_pairs):
    import ml_dtypes

    emb = np.asarray(embeddings, dtype=np.float32)
    emb_bf = emb.astype(ml_dtypes.bfloat16)
    pos = np.asarray(positive_pairs).reshape(B, N)
    neg = np.asarray(negative_pairs).reshape(B, N)

    in_maps = []
    for c in range(NCORES):
        b, h = divmod(c, 2)
        own_rows = emb_bf[b, h * HALF:(h + 1) * HALF]       # [HALF, D]
        idx = np.concatenate(
            [_wrap_idx(pos[b, h * HALF:(h + 1) * HALF]),
             _wrap_idx(neg[b, h * HALF:(h + 1) * HALF])], axis=1)
        in_maps.append({
            "table": np.ascontiguousarray(emb_bf[b]),
            "own": np.ascontiguousarray(own_rows.reshape(128, CHUNK * D)),
            "idx": np.ascontiguousarray(idx),
        })
    return in_maps


def kernel(embeddings, positive_pairs, negative_pairs):
    from concourse.bass_utils import run_bass_kernel_spmd

    nc = _get_program()
    in_maps = _shard_inputs(embeddings, positive_pairs, negative_pairs)
    res = run_bass_kernel_spmd(nc, in_maps, core_ids=list(range(NCORES)))
    total = sum(r["partial"].astype(np.float64).sum() for r in res.results)
    return np.float32(total / (B * N))
